# revision 91
# baseline (speedup 1.0000x reference)
"""Trainium2 Bass kernel for nn_Encoder (pre-norm attention + spiking FFN).

Sharding: 8 cores = 4 batches x 2 sequence halves, pure data parallel, no
collectives.  Each core receives the full 2048-token batch row with its own
query half permuted to the front (softmax over keys is permutation
invariant), computes attention for its 1024 query tokens against all 2048
keys, plus the FFN for those tokens, and returns a [1024, 512] slice.

LayerNorm affine params and linear biases are folded on the host:
  n = xhat*g + be  =>  n @ W + b == xhat @ (g[:,None]*W) + (be@W + b)
so the device only computes plain (x-mu)*rstd layernorms.

Numerics / engine strategy (validated against the reference in numpy:
rel err ~6.6e-3 vs the 2e-2 gate):
- All projection and FFN GEMMs run in fp8e4m3 with DoubleRow perf mode
  (2 k-tiles contracted per instruction).  Weights are scaled x64 on the
  host so values sit in e4m3's normal range; the 1/64 is folded into the
  psum epilogues.  Attention scores stay f16 (score range reaches ~14.5).
- Attention scores also run fp8 DoubleRow: Q/K stored in e4m3 (x8), K
  packs both heads of a pair on the 128 partitions, Q gets one plane per
  head with the other head's partitions zeroed (via per-partition
  scale/bias vectors in the Q write), and every score matmul's DR slot 1
  points at a shared zero block through a per-matmul slot stride.  Only
  the full 128-partition DR shape hits the 0.5 cyc/elem fast path on HW.
- Phase C runs as a (head, half)-unit pipeline: unit u's scores/exp are
  interleaved with unit u-1's ctx matmuls so every PE wait is satisfied
  when it executes.  (On this part the PE clock stays at the 1.2 GHz mid
  p-state throughout; the 2.4 GHz ramp never engages.)
- Softmax exp((s/64)-5) alternates engines per key-chunk-pair group
  (s,d,s,d,s,d,s,s): Scalar Exp LUT -> fp8e5m2 P, DVE Schraudolph
  bit-trick -> u8 bits of e5m2.  Both P forms feed fp8 DR ctx matmuls
  over V in e4m3 x8; the x8 V scale cancels against the 8-valued
  ones-row that accumulates Z.
- LayerNorm rstd is a batched DVE fast-inverse-sqrt (bit-trick seed via
  logical-shift + f32-value-domain C-w, 2 Newton steps) so the Scalar
  engine (which stalls behind its DMA issues at startup) is not in the
  LN dependency chain.  x moves as f16 in 4 big 3-D-AP DMAs split over
  the sync/scalar queues (~75GB/s per DMA queue is the startup wall).
- Spikes are computed as Sign(h1 + 64*b1 - 128) in {-1,+1} on the Scalar
  engine; the (s+1)/2 mapping is folded into w28 (x32) and b28 =
  colsum(w28) + 64*b2 on the host (colsum over the quantized w28 so the
  all-(-1) background cancels its quantization error exactly).
- Tail is merged per half: epilogue -> residual (gpsimd) + LN2 -> fc1 ->
  other half's epilogue ... so FFN matmuls overlap vector/scalar prep.
"""

import sys
from contextlib import ExitStack

sys.path.insert(0, "/opt/trn_rl_repo")

import ml_dtypes
import numpy as np

import concourse.bass as bass
import concourse.tile as tile
from concourse import mybir
from concourse.bass_utils import run_bass_kernel_spmd
from concourse.masks import make_identity
from concourse.vector_clock import ScopedClock, VectorClock

f32 = mybir.dt.float32
f16 = mybir.dt.float16
f8 = mybir.dt.float8e4
f8e5 = mybir.dt.float8e5
u16 = mybir.dt.uint16
u8 = mybir.dt.uint8
AF = mybir.ActivationFunctionType
ALU = mybir.AluOpType
DR = mybir.MatmulPerfMode.DoubleRow
E4NP = ml_dtypes.float8_e4m3

M, S, E, H, D, F = 4, 2048, 512, 8, 64, 2048
SQ = S // 2              # query tokens per core
N_CORES = 8
EPS = 1e-5
EC = E // 128            # 4 embed chunks
FC = F // 128            # 16 ffn chunks
TK = S // 128            # 16 key-token tiles
TQ = SQ // 128           # 8 query-token tiles
VW = D + 1               # per-head Vext width (64 v cols + ones col)
VW8 = D + 2              # fp8 Vext width (padded even for DR ldweights)
WS = 64.0                # host-side weight scale for fp8
IWS = 1.0 / WS
VS = 8.0                 # V scale inside vext (undone in epilogue)
QKS = 8.0                # Q/K fp8 store scale (scores come out x QKS^2)
CTS = 1.0 / 16.0         # ctx f16 copy scale (cancels in the Z-normalize)
NGRP = TK // 2           # key-chunk-pair groups per (head, half) unit
NSC = 5                  # groups 0..NSC-1: scalar exp->e5m2, rest: DVE Schraudolph
# f16 Schraudolph exp: bits(e^s) ~= s*1024*log2(e) + (15*1024 - 31); the -5
# softmax shift is folded in.  uint16 output clamps negatives to +0.
SCH_A = 4.0 * float(np.log2(np.e))
SCH_B = 60.0 - 0.12 - 5.0 * SCH_A


# --------------------------------------------------------------------------
# Tile framework patches for this toolchain: walrus rejects >1 sem-wait per
# instruction, so (a) the TileContext exit drain is replaced with a chain of
# single-wait SP nops, and (b) a post-pass splits any remaining multi-wait
# instruction into same-engine single-wait NoOps placed immediately before it
# (engines execute in order, so the wait point is unchanged).
# --------------------------------------------------------------------------

def _split_drain_and_barrier(self, tick_clock, wait_clock):
    g = tick_clock.global_clock
    n = len(g)
    for p in range(n):
        if g[p] > 0:
            vec = [g[p] if i == p else 0 for i in range(n)]
            nop = self.nc.sync.nop(nofuse=True, hint="split_drain")
            wait_clock.add_sem_waits(nop.ins, ScopedClock({None: VectorClock(vec)}))
    self.nc.sync.drain()
    self.nc.all_engine_barrier()
    assert self.sems is not None
    popped = self.nc._tile_sem_poison_stack.pop()
    assert popped is self._sem_poison
    self.nc.clear_and_free_semaphores(list(self.sems.allocated().values()))
    self.nc.all_engine_barrier()


tile.TileContext._drain_and_barrier = _split_drain_and_barrier


def split_multiwait(nc, limit=1):
    n_split = 0
    for fn in nc.m.functions:
        for bb in fn.blocks:
            il = bb.instructions
            out = []
            for inst in il:
                si = getattr(inst, "sync_info", None)
                waits = list(si.on_wait) if si is not None and si.on_wait else []
                if len(waits) > limit:
                    keep = waits[-limit:]
                    extra = waits[:-limit]
                    for j, w in enumerate(extra):
                        nop = mybir.InstNoOp(name=f"{inst.name}-wsplit{j}")
                        nop.engine = inst.engine
                        nop.sync_info = mybir.SyncInfo(on_wait=[w], on_update=[])
                        out.append(nop)
                        n_split += 1
                    inst.sync_info = mybir.SyncInfo(
                        on_wait=keep, on_update=list(si.on_update)
                    )
                out.append(inst)
            if len(out) != len(il):
                il[:] = out
    return n_split


# --------------------------------------------------------------------------
# Device program
# --------------------------------------------------------------------------

def build_nc(split=True):
    nc = bass.Bass()

    xin = nc.declare_dram_parameter("xin", [S, E], f16, isOutput=False)
    wq_d = nc.declare_dram_parameter("wq8", [128, EC * E], f8, isOutput=False)
    wk_d = nc.declare_dram_parameter("wk8", [128, EC * E], f8, isOutput=False)
    wv_d = nc.declare_dram_parameter("wv8", [128, EC * E], f8, isOutput=False)
    bqT_d = nc.declare_dram_parameter("bqT", [128, 2 * EC], f32, isOutput=False)
    bkT_d = nc.declare_dram_parameter("bkT", [128, EC], f32, isOutput=False)
    qsc_d = nc.declare_dram_parameter("qsc", [128, 2], f32, isOutput=False)
    bv_d = nc.declare_dram_parameter("bv", [E], f32, isOutput=False)
    w1_d = nc.declare_dram_parameter("w18", [128, EC * F], f8, isOutput=False)
    b1T_d = nc.declare_dram_parameter("b1T", [128, FC], f32, isOutput=False)
    w2_d = nc.declare_dram_parameter("w28", [128, FC * E], f8, isOutput=False)
    b2_d = nc.declare_dram_parameter("b2f", [E], f32, isOutput=False)
    out_d = nc.declare_dram_parameter("out", [SQ, E], f16, isOutput=True)

    with tile.TileContext(nc) as tc, ExitStack() as top:
        common = top.enter_context(tc.tile_pool(name="common", bufs=1))
        stats = top.enter_context(tc.tile_pool(name="stats", bufs=4))
        outp = top.enter_context(tc.tile_pool(name="outp", bufs=3))

        ident16 = common.tile([128, 128], f16, tag="ident16")
        make_identity(nc, ident16[:])

        def bcast_dma(out, src_d, n):
            src = src_d[:]
            nc.gpsimd.dma_start(
                out=out,
                in_=bass.AP(tensor=src.tensor, offset=src.offset,
                            ap=[[0, 128], [1, n]]),
            )

        # bv arrives host-prescaled by VS; broadcast straight into bv8.
        # b2rep = (colsum(w28) + 64*b2)/64, added to x1 after LN2 consumed it
        # (replaces the old ones-row bias matmul in fc2).
        bv8 = common.tile([128, E], f32, tag="bv8")
        bcast_dma(bv8[:], bv_d, E)
        b2rep = common.tile([128, E], f32, tag="b2rep")
        bcast_dma(b2rep[:], b2_d, E)
        x1 = [common.tile([128, E], f32, tag=f"x1_{t}", name=f"x1_{t}") for t in range(TQ)]
        nb5_sb = common.tile([128, 1], f32, tag="nb5")
        nc.vector.memset(nb5_sb[:], -5.0)
        # dummy activation at t=0 so the first ACT table load (~1.4us)
        # happens while the engines wait on the x DMAs anyway
        warm = common.tile([1, 1], f32, tag="warm")
        nc.scalar.activation(warm[:], nb5_sb[0:1, 0:1], AF.Identity)

        u32 = mybir.dt.uint32
        RSQRT_C = 0x5F3759DF + 1   # ~v + (C+1) == C - (u >> 1)

        def emit_stats(src_ap, mvg, i):
            st6 = stats.tile([128, 6], f32, tag="bn6")
            nc.vector.bn_stats(st6[:], src_ap)
            nc.vector.bn_aggr(mvg[:, 2 * i:2 * i + 2], st6[:])

        def emit_rsqrt(mvg, rstd, n):
            # batched DVE rsqrt(var + eps): bit-trick seed + 2 Newton steps.
            # Keeps the Scalar engine (which stalls behind its DMA issues at
            # startup) out of the layernorm dependency chain entirely.
            varv = mvg[:].rearrange("p (t c) -> p t c", c=2)[:, :, 1]
            ve = stats.tile([128, n], f32, tag="ve", name="ve")
            ta = stats.tile([128, n], f32, tag="ta", name="ta")
            tb = stats.tile([128, n], f32, tag="tb", name="tb")
            nc.vector.tensor_scalar(out=ve[:], in0=varv, scalar1=EPS,
                                    scalar2=None, op0=ALU.add)
            nc.vector.tensor_scalar(
                out=ta[:].bitcast(u32), in0=ve[:].bitcast(u32),
                scalar1=1, scalar2=None, op0=ALU.logical_shift_right,
            )
            # C - w via the f32 value path (DVE int add is f32-rounded; the
            # +/-64-ulp seed error is mopped up by the Newton steps)
            nc.vector.tensor_scalar(
                out=tb[:].bitcast(u32), in0=ta[:].bitcast(u32),
                scalar1=-1.0, scalar2=float(RSQRT_C - 1),
                op0=ALU.mult, op1=ALU.add,
            )
            y = tb
            for it in range(2):
                nc.vector.tensor_tensor(out=ta[:], in0=y[:], in1=y[:],
                                        op=ALU.mult)
                nc.vector.tensor_tensor(out=ta[:], in0=ta[:], in1=ve[:],
                                        op=ALU.mult)
                nc.vector.tensor_scalar(out=ta[:], in0=ta[:], scalar1=-0.5,
                                        scalar2=1.5, op0=ALU.mult,
                                        op1=ALU.add)
                dst = rstd if it == 1 else ve2(n)
                nc.vector.tensor_tensor(out=dst[:], in0=y[:], in1=ta[:],
                                        op=ALU.mult)
                y = dst

        def ve2(n):
            return stats.tile([128, n], f32, tag="y1", name="y1")

        def emit_norm(dst, src_ap, mvg, rstd, i, on_scalar=False):
            if on_scalar:
                # (x - mu)*rstd == x*rstd + (-mu*rstd) on the ACT engine
                nm = stats.tile([128, 1], f32, tag="nmr")
                nc.vector.tensor_scalar(
                    out=nm[:], in0=mvg[:, 2 * i:2 * i + 1],
                    scalar1=rstd[:, i:i + 1], scalar2=-1.0,
                    op0=ALU.mult, op1=ALU.mult,
                )
                nc.scalar.activation(dst, src_ap, AF.Identity,
                                     bias=nm[:], scale=rstd[:, i:i + 1])
            else:
                nc.vector.tensor_scalar(
                    out=dst,
                    in0=src_ap,
                    scalar1=mvg[:, 2 * i:2 * i + 1],
                    scalar2=rstd[:, i:i + 1],
                    op0=ALU.subtract,
                    op1=ALU.mult,
                )

        with ExitStack() as attn:
            resA = attn.enter_context(tc.tile_pool(name="resA", bufs=1))
            tempA = attn.enter_context(tc.tile_pool(name="tempA", bufs=3))
            projsc = ExitStack()
            ps_tr = projsc.enter_context(
                tc.tile_pool(name="ps_tr", bufs=2, space="PSUM")
            )
            ps_proj = projsc.enter_context(
                tc.tile_pool(name="ps_proj", bufs=2, space="PSUM")
            )

            # ---- DMA schedule: x tiles split across the sync + scalar
            # queues (~128GB/s each) so the first layernorms start ASAP;
            # weights go on the gpsimd queue in parallel so the first
            # projections (~8us) aren't starved behind the 4MB of x. ----
            xbig = resA.tile([128, TK * E], f16, tag="xbig", name="xbig")
            xall = [xbig[:, t * E:(t + 1) * E] for t in range(TK)]
            wq_sb = resA.tile([128, EC * E], f8, tag="wq8")
            wk_sb = resA.tile([128, EC * E], f8, tag="wk8")
            wv_sb = resA.tile([128, EC * E], f8, tag="wv8")
            bqT = resA.tile([128, 2 * EC], f32, tag="bqT")
            bkT = resA.tile([128, EC], f32, tag="bkT")
            qsc = resA.tile([128, 2], f32, tag="qsc")
            nc.gpsimd.dma_start(wk_sb[:], wk_d[:])
            nc.gpsimd.dma_start(wq_sb[:], wq_d[:])
            nc.gpsimd.dma_start(wv_sb[:], wv_d[:])
            nc.gpsimd.dma_start(bkT[:], bkT_d[:])
            nc.gpsimd.dma_start(bqT[:], bqT_d[:])
            nc.gpsimd.dma_start(qsc[:], qsc_d[:])
            # x in 4 big quarter-DMAs (3-D APs: per-partition 4KB runs) —
            # 16 small per-tile DMAs only reach ~75GB/s/queue, the big
            # transfers land all of x by ~12us.
            xi = xin[:]
            for qtr in range(4):
                q = nc.sync if qtr % 2 == 0 else nc.scalar
                q.dma_start(
                    xbig[:, qtr * 4 * E:(qtr + 1) * 4 * E],
                    bass.AP(tensor=xi.tensor, offset=xi.offset + qtr * 512 * E,
                            ap=[[E, 128], [128 * E, 4], [1, E]]),
                )

            # ---- phase A+B interleaved: per 512-token group, LN1 ->
            # transpose -> K/Q/V projections (fp8 DoubleRow).  Projections
            # only need this group's tokens, so the PE starts after the
            # first group's layernorms instead of after all 16. ----
            xhatT8 = resA.tile([128, EC * S], f8, tag="xhT8")
            xhT_r = xhatT8[:].rearrange("p (k t) -> p k t", k=EC)
            wq_r = wq_sb[:].rearrange("p (k e) -> p k e", k=EC)
            wk_r = wk_sb[:].rearrange("p (k e) -> p k e", k=EC)
            wv_r = wv_sb[:].rearrange("p (k e) -> p k e", k=EC)

            # Q/K stores in fp8e4m3 (x8 scale) shaped so the score matmuls run
            # the STANDARD 128-partition DoubleRow shape (the only one the HW
            # runs at 0.5 cyc/out-elem; 64-partition DR falls off the fast
            # path).  K keeps both heads of a pair packed on the 128
            # partitions; Q gets one plane per head with the other head's 64
            # partitions zeroed (via per-partition scale/bias vectors in the
            # Q write itself), so the cross-head products vanish via the
            # Q-side zeros.  The DR slot-1 operand of every score matmul
            # points at a SHARED zero block at the end of each tile via a
            # per-matmul slot stride, so only ~3KB/lane of zeros exist.
            kT8 = [resA.tile([128, S + 128], f8, tag=f"kT8_{k}",
                             name=f"kT8_{k}") for k in range(EC)]
            qT8 = [resA.tile([128, 2 * SQ + 512], f8, tag=f"qT8_{k}",
                             name=f"qT8_{k}") for k in range(EC)]
            for k in range(EC):
                nc.vector.memset(kT8[k][:, S:S + 128], 0.0)
                (nc.vector if k % 2 == 0 else nc.gpsimd).memset(
                    qT8[k][:, 2 * SQ:2 * SQ + 512], 0.0)

            def dr_slot_ap(t_ap, off, width, zoff):
                # [128, 2, width] DR operand: slot 0 = cols off..off+width,
                # slot 1 = the zero block at zoff (per-AP slot stride).
                return bass.AP(tensor=t_ap.tensor, offset=t_ap.offset + off,
                               ap=[list(t_ap.ap[0]), [zoff - off, 2],
                                   [1, width]])
            # V x8 in e4m3; ones col = 8 so the scale cancels in ctx/Z.
            vext8 = resA.tile([128, TK * H * VW8], f8, tag="vext8")
            vext8_r = vext8[:].rearrange("p (t h c) -> p t h c", t=TK, h=H)

            for g in range(S // 512):
                mvg = stats.tile([128, 8], f32, tag="mvg", name="mvg")
                rstd4 = stats.tile([128, 4], f32, tag="rstd4", name="rstd4")
                for i, t in enumerate(range(4 * g, 4 * g + 4)):
                    emit_stats(xall[t][:], mvg, i)
                emit_rsqrt(mvg, rstd4, 4)
                for i, t in enumerate(range(4 * g, 4 * g + 4)):
                    xh = tempA.tile([128, E], f16, tag="xh1")
                    emit_norm(xh[:], xall[t][:], mvg, rstd4, i,
                              on_scalar=(t % 2 == 1))
                    tpq = ps_tr.tile([128, E], f16, tag="t16")
                    for k in range(EC):
                        nc.tensor.transpose(
                            tpq[:, k * 128:(k + 1) * 128],
                            xh[:, k * 128:(k + 1) * 128], ident16[:])
                    if t % 2 == 0:
                        nc.scalar.copy(
                            xhT_r[:, :, t * 128:(t + 1) * 128],
                            tpq[:].rearrange("p (k c) -> p k c", k=EC))
                    else:
                        nc.vector.tensor_copy(
                            xhT_r[:, :, t * 128:(t + 1) * 128],
                            tpq[:].rearrange("p (k c) -> p k c", k=EC))
                for dc in range(EC):
                    ps = ps_proj.tile([128, 512], f32, tag="mm512")
                    for jp in range(EC // 2):
                        nc.tensor.matmul(
                            ps[:],
                            wk_r[:, 2 * jp:2 * jp + 2, dc * 128:(dc + 1) * 128],
                            xhT_r[:, 2 * jp:2 * jp + 2, g * 512:(g + 1) * 512],
                            start=(jp == 0),
                            stop=(jp == EC // 2 - 1),
                            perf_mode=DR,
                        )
                    nc.scalar.activation(
                        kT8[dc][:, g * 512:(g + 1) * 512], ps[:], AF.Identity,
                        bias=bkT[:, dc:dc + 1], scale=QKS * IWS,
                    )
                    if g < SQ // 512:
                        ps = ps_proj.tile([128, 512], f32, tag="mm512")
                        for jp in range(EC // 2):
                            nc.tensor.matmul(
                                ps[:],
                                wq_r[:, 2 * jp:2 * jp + 2, dc * 128:(dc + 1) * 128],
                                xhT_r[:, 2 * jp:2 * jp + 2, g * 512:(g + 1) * 512],
                                start=(jp == 0),
                                stop=(jp == EC // 2 - 1),
                                perf_mode=DR,
                            )
                        # full-width writes; the per-partition scale/bias
                        # vectors zero the other head's 64 partitions.
                        nc.scalar.activation(
                            qT8[dc][:, g * 512:(g + 1) * 512],
                            ps[:], AF.Identity,
                            bias=bqT[:, dc:dc + 1], scale=qsc[:, 0:1],
                        )
                        nc.vector.tensor_scalar(
                            out=qT8[dc][:, SQ + g * 512:SQ + (g + 1) * 512],
                            in0=ps[:],
                            scalar1=qsc[:, 1:2],
                            scalar2=bqT[:, EC + dc:EC + dc + 1],
                            op0=ALU.mult,
                            op1=ALU.add,
                        )
                for t in range(4 * g, 4 * g + 4):
                    ps = ps_proj.tile([128, 512], f32, tag="mm512", name="vps")
                    for jp in range(EC // 2):
                        nc.tensor.matmul(
                            ps[:],
                            xhT_r[:, 2 * jp:2 * jp + 2, t * 128:(t + 1) * 128],
                            wv_r[:, 2 * jp:2 * jp + 2, :],
                            start=(jp == 0),
                            stop=(jp == EC // 2 - 1),
                            perf_mode=DR,
                        )
                    vv, vwidth = vext8_r[:, t], VW8
                    nc.vector.scalar_tensor_tensor(
                        out=vv[:, :, 0:D],
                        in0=ps[:].rearrange("p (h c) -> p h c", c=D),
                        scalar=VS * IWS,
                        in1=bv8[:].rearrange("p (h c) -> p h c", c=D),
                        op0=ALU.mult,
                        op1=ALU.add,
                    )
                    nc.vector.memset(vv[:, :, D:vwidth], VS)

            projsc.close()

            # ---- phase C: attention, (head, half) unit pipeline ----
            # Unit u's scores/exp interleave with unit u-1's ctx matmuls so
            # every PE wait is satisfied by the time it executes, keeping the
            # PE stream unbroken (p-state ramps to max).  exp runs on the
            # Scalar engine for groups 0..NSC-1 (e5m2 out, fp8 DR ctx) and as
            # a Schraudolph bit-trick on the Vector engine for the rest
            # (uint16 out bitcast to f16, f16 ctx).
            att_sb = resA.tile([128, TQ * E], f32, tag="att")
            att_r = att_sb[:].rearrange("p (q e) -> p q e", q=TQ)
            # prefetch the Exp ACT table during phase B so the load isn't on
            # unit 0's first-exp critical path
            warm2 = resA.tile([1, 1], f32, tag="warm2")
            nc.scalar.activation(warm2[:], nb5_sb[0:1, 0:1], AF.Exp)
            # half-major order: units 0..H-1 cover query half 0, so its
            # attention completes early and the FFN for half 0 can overlap
            # the tail of phase C / half 1's epilogue.
            units = [(h, half) for half in range(SQ // 512) for h in range(H)]
            ctxs_all = [resA.tile([VW, 512], f16, tag=f"cx{u}", name=f"cx{u}")
                        for u in range(len(units))]

            def copy_ctx(dst, src, on_scalar=False):
                # f32 psum -> f16 SBUF, scaled down so Z can't overflow f16;
                # the scale cancels in the ctx/Z normalize.
                if on_scalar:
                    nc.scalar.activation(dst, src, AF.Identity, scale=CTS)
                else:
                    nc.vector.tensor_scalar(
                        out=dst, in0=src, scalar1=CTS, scalar2=None,
                        op0=ALU.mult)

            with ExitStack() as cmain:
                ps_st = cmain.enter_context(
                    tc.tile_pool(name="ps_st", bufs=3, space="PSUM")
                )
                ps_ctx = cmain.enter_context(
                    tc.tile_pool(name="ps_ctx", bufs=2, space="PSUM")
                )
                p5p = cmain.enter_context(
                    tc.tile_pool(name="p5p", bufs=NGRP))
                pup = cmain.enter_context(
                    tc.tile_pool(name="pup", bufs=NGRP))

                def emit_ctx_group(prev, g):
                    h, half, ctx_t, Ps = prev
                    kind, P = Ps[g]
                    p_ap = (P[:] if kind == "e5"
                            else P[:].bitcast(f8e5))
                    nc.tensor.matmul(
                        ctx_t[:],
                        vext8_r[:, 2 * g:2 * g + 2, h],
                        p_ap.rearrange("p (two n) -> p two n", two=2),
                        start=(g == 0),
                        stop=(g == NGRP - 1),
                        perf_mode=DR,
                        skip_group_check=True,
                    )

                prev = None
                for u, (h, half) in enumerate(units):
                    dc, row = h // 2, (h % 2) * D
                    ctx_t = ps_ctx.tile([VW8, 512], f32, tag="ctx", name=f"ctx{u}")
                    Ps = []
                    for g in range(NGRP):
                        st = ps_st.tile([128, 1024], f32, tag="st", name="st")
                        for j in range(2):
                            kc = 2 * g + j
                            nc.tensor.matmul(
                                st[:, j * 512:(j + 1) * 512],
                                dr_slot_ap(kT8[dc][:], kc * 128, 128, S),
                                dr_slot_ap(qT8[dc][:],
                                           (h % 2) * SQ + half * 512, 512,
                                           2 * SQ),
                                start=True,
                                stop=True,
                                perf_mode=DR,
                            )
                        if g % 2 == 0 or g == 7:
                            P = p5p.tile([128, 1024], f8e5, tag="p5", name="p5")
                            nc.scalar.activation(P[:], st[:], AF.Exp,
                                                 bias=nb5_sb[:],
                                                 scale=1.0 / (QKS * QKS))
                            Ps.append(("e5", P))
                        else:
                            P = pup.tile([128, 1024], u8, tag="pu", name="pu")
                            nc.vector.tensor_scalar(
                                out=P[:],
                                in0=st[:],
                                scalar1=SCH_A / (QKS * QKS),
                                scalar2=SCH_B,
                                op0=ALU.mult,
                                op1=ALU.add,
                            )
                            Ps.append(("sch", P))
                        if prev is not None:
                            emit_ctx_group(prev, g)
                    if prev is not None:
                        copy_ctx(ctxs_all[u - 1][:], prev[2][0:VW, :],
                                 on_scalar=(u - 1 >= len(units) - 2))
                    prev = (h, half, ctx_t, Ps)
                for g in range(NGRP):
                    emit_ctx_group(prev, g)
                copy_ctx(ctxs_all[len(units) - 1][:], prev[2][0:VW, :],
                         on_scalar=True)
                # pull the Sign ACT-table load off the fc1 critical path:
                # load it during the trailing epilogue window where Scalar is
                # idle.  Reading the last ctx copy pins this AFTER phase C's
                # final Exp (a dep-free dummy could be scheduled early and
                # evict the Exp table mid-attention).
                warm3 = resA.tile([1, 1], f32, tag="warm3")
                nc.scalar.activation(
                    warm3[:], ctxs_all[len(units) - 1][0:1, 0:1], AF.Sign)

            # ---- merged tail: per half, epilogue -> residual+LN2 -> FFN.
            # FFN matmuls for half 0 overlap the vector/scalar-heavy
            # epilogue + layernorm prep for half 1. ----
            NJ = SQ // 512 * 2
            with ExitStack() as tail:
                resB = tail.enter_context(tc.tile_pool(name="resB", bufs=1))
                tempB = tail.enter_context(tc.tile_pool(name="tempB", bufs=3))
                ps_tp = tail.enter_context(
                    tc.tile_pool(name="ps_tp", bufs=2, space="PSUM"))
                ps_tr2 = tail.enter_context(
                    tc.tile_pool(name="ps_tr2", bufs=2, space="PSUM"))
                ps_h1 = tail.enter_context(
                    tc.tile_pool(name="ps_h1", bufs=2, space="PSUM"))
                ps_f2 = tail.enter_context(
                    tc.tile_pool(name="ps_f2", bufs=2, space="PSUM"))

                w1_sb = resB.tile([128, EC * F], f8, tag="w18")
                nc.sync.dma_start(w1_sb[:], w1_d[:])
                b1T = resB.tile([128, FC], f32, tag="b1T")
                nc.sync.dma_start(b1T[:], b1T_d[:])
                w2_sb = resB.tile([128, FC * E], f8, tag="w28")
                nc.sync.dma_start(w2_sb[:], w2_d[:])
                w1_r = w1_sb[:].rearrange("p (k f) -> p k f", k=EC)
                w2_r = w2_sb[:].rearrange("p (c e) -> p c e", c=FC)
                xh2T8 = resB.tile([128, EC * SQ], f8, tag="x2T8")
                xh2_r = xh2T8[:].rearrange("p (k t) -> p k t", k=EC)
                spkT = resB.tile([128, FC * SQ], f8, tag="spkT")
                spk_r = spkT[:].rearrange("p (c t) -> p c t", c=FC)

                def emit_epi(lo, hi):
                    # transpose ctx^T, scale by 1/(8Z) (the x8 V scale
                    # cancels against the 8-valued ones row)
                    for u in range(lo, hi):
                        h, half = units[u]
                        tpb = ps_tp.tile([128, NJ * (VW + 1)], f16, tag="tpb",
                                         name="tpb")
                        tpb_r = tpb[:].rearrange("p (q c) -> p q c", q=NJ)
                        for j in range(NJ):
                            nc.tensor.transpose(
                                tpb_r[:, j, 0:VW],
                                ctxs_all[u][:, j * 128:(j + 1) * 128],
                                ident16[0:VW, 0:VW]
                            )
                        rec = stats.tile([128, NJ], f32, tag="zrec")
                        nc.vector.reciprocal(
                            rec[:], tpb_r[:, :, D:D + 1].squeeze(2))
                        nc.vector.tensor_tensor(
                            out=att_r[:, half * NJ:(half + 1) * NJ,
                                      h * D:(h + 1) * D],
                            in0=tpb_r[:, :, 0:D],
                            in1=rec[:].unsqueeze(2).broadcast_to(
                                [128, NJ, D]),
                            op=ALU.mult,
                        )

                def emit_d_ln2(qcs):
                    qcs = list(qcs)
                    mvg = stats.tile([128, 2 * len(qcs)], f32, tag="mvg2",
                                     name="mvg2")
                    rstdn = stats.tile([128, len(qcs)], f32, tag="rstd2",
                                       name="rstd2")
                    for i, qc in enumerate(qcs):
                        addq = nc.vector if qc % 2 == 0 else nc.gpsimd
                        addq.tensor_add(x1[qc][:], xall[qc][:],
                                        att_r[:, qc])
                        emit_stats(x1[qc][:], mvg, i)
                    emit_rsqrt(mvg, rstdn, len(qcs))
                    for i, qc in enumerate(qcs):
                        xh2 = tempB.tile([128, E], f16, tag="xh2")
                        emit_norm(xh2[:], x1[qc][:], mvg, rstdn, i,
                                  on_scalar=(qc % 2 == 0))
                        # after LN2 consumed x1, fold the fc2 bias into the
                        # residual (replaces the old ones-row bias matmul)
                        addq2 = nc.gpsimd if qc % 2 == 0 else nc.vector
                        addq2.tensor_add(x1[qc][:], x1[qc][:], b2rep[:])
                        tpq2 = ps_tr2.tile([128, E], f16, tag="t16b")
                        for k in range(EC):
                            nc.tensor.transpose(
                                tpq2[:, k * 128:(k + 1) * 128],
                                xh2[:, k * 128:(k + 1) * 128], ident16[:]
                            )
                        nc.scalar.copy(
                            xh2_r[:, :, qc * 128:(qc + 1) * 128],
                            tpq2[:].rearrange("p (k c) -> p k c", k=EC))

                def emit_fc1(half):
                    for c in range(FC):
                        ps = ps_h1.tile([128, 512], f32, tag="h1")
                        for jp in range(EC // 2):
                            nc.tensor.matmul(
                                ps[:],
                                w1_r[:, 2 * jp:2 * jp + 2,
                                     c * 128:(c + 1) * 128],
                                xh2_r[:, 2 * jp:2 * jp + 2,
                                      half * 512:(half + 1) * 512],
                                start=(jp == 0),
                                stop=(jp == EC // 2 - 1),
                                perf_mode=DR,
                            )
                        # spike in {-1,+1} via Sign on the Scalar engine;
                        # the (s+1)/2 mapping is folded into w28 (x32) and
                        # b28 on the host.  b1T holds 64*b1 - 128.
                        nc.scalar.activation(
                            spk_r[:, c, half * 512:(half + 1) * 512],
                            ps[:],
                            AF.Sign,
                            bias=b1T[:, c:c + 1],
                        )

                def emit_fc2(half):
                    for qc in range(half * (TQ // 2), (half + 1) * (TQ // 2)):
                        ps = ps_f2.tile([128, E], f32, tag="mm512")
                        for jp in range(FC // 2):
                            nc.tensor.matmul(
                                ps[:],
                                spk_r[:, 2 * jp:2 * jp + 2,
                                      qc * 128:(qc + 1) * 128],
                                w2_r[:, 2 * jp:2 * jp + 2, :],
                                start=(jp == 0),
                                stop=(jp == FC // 2 - 1),
                                perf_mode=DR,
                            )
                        ot = outp.tile([128, E], f16, tag="ot")
                        nc.vector.scalar_tensor_tensor(
                            out=ot[:],
                            in0=ps[:],
                            scalar=IWS,
                            in1=x1[qc][:],
                            op0=ALU.mult,
                            op1=ALU.add,
                        )
                        oq = (nc.sync, nc.scalar, nc.gpsimd)[qc % 3]
                        oq.dma_start(
                            out_d[qc * 128:(qc + 1) * 128, :], ot[:])

                nu = len(units)
                emit_epi(0, nu // 2)
                emit_d_ln2(range(0, TQ // 2))
                emit_fc1(0)
                emit_epi(nu // 2, nu)
                emit_d_ln2(range(TQ // 2, TQ))
                emit_fc2(0)
                emit_fc1(1)
                emit_fc2(1)

    if split:
        split_multiwait(nc)
    return nc


_NC = None


def _get_nc():
    global _NC
    if _NC is None:
        _NC = build_nc()
    return _NC


# --------------------------------------------------------------------------
# Host wrapper
# --------------------------------------------------------------------------

def _to_f8(a):
    return np.ascontiguousarray(a).astype(E4NP)


def _prep_weights(inputs):
    f = lambda k: np.asarray(inputs[k], np.float32)
    g1, be1 = f("g1"), f("be1")
    g2, be2 = f("g2"), f("be2")
    wq, wk, wv = f("wq"), f("wk"), f("wv")
    bq, bk, bv = f("bq"), f("bk"), f("bv")
    w1, b1 = f("w1"), f("b1")
    w2, b2 = f("w2"), f("b2")

    wq_e = wq * g1[:, None] * WS
    wk_e = wk * g1[:, None] * WS
    wv_e = wv * g1[:, None] * WS
    bq_e = bq + be1 @ wq
    bk_e = bk + be1 @ wk
    bv_e = bv + be1 @ wv
    w1_e = w1 * g2[:, None] * WS
    b1_e = (b1 + be2 @ w1) * WS - 128.0   # Sign threshold: h1 >= 2
    # spike = (sign+1)/2 folded into fc2: 64*(spk01@w2 + b2) =
    #   spk_pm1 @ (32*w2) + (colsum(32*w2) + 64*b2).
    # colsum MUST be over the fp8-quantized weights so the all-(-1)
    # background cancels its quantization error exactly.
    w2_e = (w2 * (WS / 2.0)).astype(E4NP).astype(np.float32)
    b2_e = w2_e.sum(axis=0) + b2 * WS

    # [e_in, e_out] -> [128, ec, e_out] with e_in = ec*128 + p
    def chunked(w, nin, nout):
        return w.reshape(nin // 128, 128, nout).transpose(1, 0, 2).reshape(
            128, (nin // 128) * nout)

    # bqT planes: plane hh carries head 2dc+hh's bias on its own 64
    # partitions and 0 on the other head's, matching the qsc scale vectors
    # that zero the complementary partitions in the Q writes.
    bq_rs = np.ascontiguousarray(bq_e.reshape(EC, 128).T) * QKS  # [128, EC]
    bq_pad = np.zeros((128, 2 * EC), np.float32)
    bq_pad[0:64, 0:EC] = bq_rs[0:64]
    bq_pad[64:128, EC:2 * EC] = bq_rs[64:128]
    qsc = np.zeros((128, 2), np.float32)
    qsc[0:64, 0] = QKS * IWS
    qsc[64:128, 1] = QKS * IWS

    return {
        "wq8": _to_f8(chunked(wq_e, E, E)),
        "wk8": _to_f8(chunked(wk_e, E, E)),
        "wv8": _to_f8(chunked(wv_e, E, E)),
        "bqT": bq_pad,
        "bkT": np.ascontiguousarray(bk_e.reshape(EC, 128).T) * QKS,
        "qsc": qsc,
        "bv": bv_e * VS,
        "w18": _to_f8(chunked(w1_e, E, F)),
        "b1T": np.ascontiguousarray(b1_e.reshape(FC, 128).T),
        "w28": _to_f8(chunked(w2_e, F, E)),
        "b2f": (b2_e * IWS).astype(np.float32),
    }


def _run(inputs, **spmd_kwargs):
    x = np.asarray(inputs["x"], np.float32).astype(np.float16)
    w = _prep_weights(inputs)
    in_maps = []
    for c in range(N_CORES):
        b, h = c // 2, c % 2
        xq = x[b, h * SQ:(h + 1) * SQ]
        xo = x[b, (1 - h) * SQ:(2 - h) * SQ]
        m = dict(w)
        m["xin"] = np.ascontiguousarray(np.concatenate([xq, xo], axis=0))
        in_maps.append(m)
    res = run_bass_kernel_spmd(_get_nc(), in_maps, list(range(N_CORES)), **spmd_kwargs)
    out = np.empty((M, S, E), np.float32)
    for c in range(N_CORES):
        b, h = c // 2, c % 2
        out[b, h * SQ:(h + 1) * SQ] = res.results[c]["out"]
    return out, res


def kernel(**inputs):
    try:
        out, _ = _run(inputs)
    except Exception:
        # transient device hiccups (NRT exec-unit resets) recover on retry
        out, _ = _run(inputs)
    return out

